# revision 1
# baseline (speedup 1.0000x reference)
"""BiDecoder edge kernel for Trainium2, 8-core SPMD.

out[e, r] = (u[edge_user[e]] @ W[r].T + b[r]) . i[edge_item[e]]
          = sum_j i_row[j] * ( sum_k W[r,j,k] u_row[k] + b[r,j] )

Distribution: edges sharded by contiguous user ranges across 8 cores; each
core receives its own user-table slice (u_shard) plus the full item table.

Per-core algorithm (all data-dependent access via int16 dma_gather):
  - host sorts the shard by (item_window, user) and packs 128-edge tiles with
    <= NQ unique users each;
  - item rows are bulk-gathered per edge (window-relative int16 idx);
  - unique user rows are gathered once per tile (shard-relative idx);
  - per 8-tile group, PE computes C[j,(r,q)] = W_r @ U_uniq^T + b_r (bias
    folded via a K=1 accumulating matmul);
  - per tile, PE computes G[e,(r,q)] = I_tile^T . C  (I^T stationary);
  - DVE selects q = pos(e) with a batched one-hot multiply + reduce;
  - PE transposes output batches so the DRAM write is dense.
"""
import numpy as np
from contextlib import ExitStack

import concourse.bacc as bacc
import concourse.bass as bass
import concourse.tile as tile
from concourse import mybir
from concourse.bass_utils import run_bass_kernel_spmd
from concourse.masks import make_identity

F32 = mybir.dt.float32
I16 = mybir.dt.int16

N_CORES = 8
D = 64
R = 5
NQ = 16            # max unique users per 128-edge tile
CGRP = 8           # tiles per C-group (CGRP*NQ = 128 unique rows)
TBATCH = 8         # tiles per select/output batch
NT_ALIGN = 8       # lcm(CGRP, TBATCH)
BLK_TILES = 64     # tiles per bulk dma_gather block (8192 edges)
UBLK = 2048        # unique rows per uniq dma_gather block
IWIN = 32768       # item-window size (int16 index limit)


# ----------------------------------------------------------------- host prep

def _prep_core(users, items, eidx):
    """Tile-pack one core's edges. Edges are sorted here by
    (item_window, user). Returns dict of per-tile arrays."""
    iwin_all = items // IWIN
    order = np.lexsort((users, iwin_all))
    users = users[order]; items = items[order]; eidx = eidx[order]
    iwin_all = iwin_all[order]

    n = len(users)
    newgrp = np.ones(n, dtype=bool)
    if n > 1:
        newgrp[1:] = (users[1:] != users[:-1]) | (iwin_all[1:] != iwin_all[:-1])
    grp_starts = np.nonzero(newgrp)[0]
    grp_ends = np.append(grp_starts[1:], n)

    tile_win, slot_item, slot_eidx, slot_pos, tile_uniq = [], [], [], [], []
    ci, ce, cp, cuq = [], [], [], []
    cw = -1

    def flush():
        nonlocal ci, ce, cp, cuq
        if not ci:
            return
        pi = ci[0]
        while len(ci) < 128:
            ci.append(pi); ce.append(-1); cp.append(0)
        uq = list(cuq)
        while len(uq) < NQ:
            uq.append(uq[0])
        tile_win.append(cw)
        slot_item.append(ci); slot_eidx.append(ce)
        slot_pos.append(cp); tile_uniq.append(uq)
        ci, ce, cp, cuq = [], [], [], []

    for gs, ge in zip(grp_starts, grp_ends):
        w = int(iwin_all[gs]); u = int(users[gs])
        pos = int(gs)
        while pos < ge:
            if ci and (cw != w or len(ci) >= 128 or
                       (u not in cuq and len(cuq) >= NQ)):
                flush()
            cw = w
            if u not in cuq:
                cuq.append(u)
            p = cuq.index(u)
            take = min(int(ge) - pos, 128 - len(ci))
            ci.extend(items[pos:pos + take].tolist())
            ce.extend(eidx[pos:pos + take].tolist())
            cp.extend([p] * take)
            pos += take
    flush()

    return {
        "tile_win": np.array(tile_win, dtype=np.int64),
        "slot_item": np.array(slot_item, dtype=np.int64),
        "slot_eidx": np.array(slot_eidx, dtype=np.int64),
        "slot_pos": np.array(slot_pos, dtype=np.int64),
        "tile_uniq": np.array(tile_uniq, dtype=np.int64),
    }


def _wrap16(idx_i16):
    """[n] int16 (n%16==0) -> [128, n//16]: idx j -> row j%16, col j//16,
    16-row block replicated to 128 partitions."""
    n = len(idx_i16)
    w = idx_i16.reshape(n // 16, 16).T
    return np.ascontiguousarray(np.tile(w, (8, 1)))


def host_prep(u_features, i_features, edge_user, edge_item):
    E = len(edge_user)
    NU = u_features.shape[0]
    NI = i_features.shape[0]
    n_iwin = (NI + IWIN - 1) // IWIN

    eu = np.asarray(edge_user, dtype=np.int64)
    ei = np.asarray(edge_item, dtype=np.int64)

    # shard by contiguous user ranges with ~equal edge counts
    order_u = np.argsort(eu, kind="stable")
    eus = eu[order_u]
    bounds = [0]
    for c in range(1, N_CORES):
        t = (E * c) // N_CORES
        while 0 < t < E and eus[t] == eus[t - 1]:
            t += 1
        bounds.append(min(t, E))
    bounds.append(E)

    cores = []
    for c in range(N_CORES):
        sl = order_u[bounds[c]:bounds[c + 1]]
        cores.append(_prep_core(eu[sl], ei[sl], sl.copy()))

    ushard_start, ushard_len = [], []
    for c in range(N_CORES):
        uq = cores[c]["tile_uniq"]
        lo = int(uq.min()) if uq.size else 0
        hi = int(uq.max()) + 1 if uq.size else 1
        ushard_start.append(lo); ushard_len.append(hi - lo)
    MAXU = max(ushard_len)
    assert MAXU <= 32768, f"user shard {MAXU} exceeds int16 range"

    # equalize per-item-window tile counts across cores
    sec_tiles = np.zeros((N_CORES, n_iwin), dtype=np.int64)
    for c in range(N_CORES):
        for w in range(n_iwin):
            sec_tiles[c, w] = int((cores[c]["tile_win"] == w).sum())
    sec_pad = ((sec_tiles.max(axis=0) + CGRP - 1) // CGRP) * CGRP
    sec_pad = np.maximum(sec_pad, CGRP)
    while int(sec_pad.sum()) % NT_ALIGN != 0:
        sec_pad[-1] += CGRP
    NT = int(sec_pad.sum())
    n_bulk_blocks = [(int(s) + BLK_TILES - 1) // BLK_TILES for s in sec_pad]
    sec_blk_tiles = [n_bulk_blocks[w] * BLK_TILES for w in range(n_iwin)]
    NT_U = ((NT * NQ + UBLK - 1) // UBLK) * UBLK // NQ  # tiles covered by ublks

    in_maps, reasm = [], []
    for c in range(N_CORES):
        d = cores[c]
        slot_item_p = np.zeros((NT, 128), dtype=np.int64)
        slot_pos_p = np.zeros((NT, 128), dtype=np.int64)
        tile_uniq_p = np.zeros((NT, NQ), dtype=np.int64)
        slot_eidx_p = np.full((NT, 128), -1, dtype=np.int64)
        t_out = 0
        for w in range(n_iwin):
            sel = np.nonzero(d["tile_win"] == w)[0]
            k = len(sel); spw = int(sec_pad[w])
            if k:
                slot_item_p[t_out:t_out + k] = d["slot_item"][sel]
                slot_pos_p[t_out:t_out + k] = d["slot_pos"][sel]
                tile_uniq_p[t_out:t_out + k] = d["tile_uniq"][sel]
                slot_eidx_p[t_out:t_out + k] = d["slot_eidx"][sel]
            slot_item_p[t_out + k:t_out + spw] = w * IWIN
            tile_uniq_p[t_out + k:t_out + spw] = ushard_start[c]
            t_out += spw
        assert t_out == NT

        bulk_parts = []
        t0 = 0
        for w in range(n_iwin):
            spw = int(sec_pad[w])
            rel = (slot_item_p[t0:t0 + spw] - w * IWIN).astype(np.int16).ravel()
            pad = np.zeros(128 * (sec_blk_tiles[w] - spw), dtype=np.int16)
            bulk_parts.append(np.concatenate([rel, pad]))
            t0 += spw
        bulk_wrapped = _wrap16(np.concatenate(bulk_parts))

        uq = np.zeros((NT_U, NQ), dtype=np.int64)
        uq[:NT] = tile_uniq_p - ushard_start[c]
        uniq_wrapped = _wrap16(uq.ravel().astype(np.int16))

        pos_f32 = np.ascontiguousarray(slot_pos_p.T.astype(np.float32))

        ush = np.zeros((MAXU, D), dtype=np.float32)
        n_avail = min(MAXU, NU - ushard_start[c])
        ush[:n_avail] = u_features[ushard_start[c]:ushard_start[c] + n_avail]

        in_maps.append({
            "u_shard": ush,
            "i_tab": np.ascontiguousarray(i_features, dtype=np.float32),
            "bulk_idx": bulk_wrapped,
            "uniq_idx": uniq_wrapped,
            "pos": pos_f32,
        })
        reasm.append(slot_eidx_p)

    meta = {
        "NT": NT, "NT_U": NT_U, "n_iwin": n_iwin,
        "n_bulk_blocks": n_bulk_blocks, "sec_blk_tiles": sec_blk_tiles,
        "sec_pad": [int(x) for x in sec_pad], "MAXU": MAXU, "NI": NI, "E": E,
    }
    return in_maps, reasm, meta


# ------------------------------------------------------------ device program

def _bc_mid(ap, size, axis):
    """Insert a stride-0 dim of `size` at free-dim position `axis` (counting
    the partition dim as 0)."""
    lst = [list(x) for x in ap.ap]
    lst.insert(axis, [0, size])
    return bass.AP(ap.tensor, ap.offset, [tuple(x) for x in lst])


def build_program(meta):
    NT, NT_U = meta["NT"], meta["NT_U"]
    n_iwin = meta["n_iwin"]
    MAXU, NI = meta["MAXU"], meta["NI"]
    n_bulk_blocks = meta["n_bulk_blocks"]
    sec_pad = meta["sec_pad"]
    NBULKC = sum(meta["sec_blk_tiles"]) * 8   # bulk_idx columns
    n_ublk = NT_U * NQ // UBLK

    nc = bacc.Bacc("TRN2", target_bir_lowering=False, debug=False,
                   num_devices=N_CORES, num_swdge_queues=4)

    u_shard = nc.dram_tensor("u_shard", [MAXU, D], F32, kind="ExternalInput").ap()
    i_tab = nc.dram_tensor("i_tab", [NI, D], F32, kind="ExternalInput").ap()
    bulk_idx = nc.dram_tensor("bulk_idx", [128, NBULKC], I16,
                              kind="ExternalInput").ap()
    uniq_idx = nc.dram_tensor("uniq_idx", [128, NT_U * NQ // 16], I16,
                              kind="ExternalInput").ap()
    pos_in = nc.dram_tensor("pos", [128, NT], F32, kind="ExternalInput").ap()
    wb = nc.dram_tensor("wb", [65, R, 64], F32, kind="ExternalInput").ap()
    bvec = nc.dram_tensor("bvec", [1, R, 64], F32, kind="ExternalInput").ap()
    ones_in = nc.dram_tensor("ones_in", [1, 128], F32, kind="ExternalInput").ap()
    iota_in = nc.dram_tensor("iota16", [128, NQ], F32, kind="ExternalInput").ap()
    out_dev = nc.dram_tensor("out_dev", [NT * R, 128], F32,
                             kind="ExternalOutput").ap()

    with tile.TileContext(nc) as tc, ExitStack() as ctx:
        cpool = ctx.enter_context(tc.tile_pool(name="const", bufs=1))
        bulkpool = ctx.enter_context(tc.tile_pool(name="bulk", bufs=3))
        itpool = ctx.enter_context(tc.tile_pool(name="it", bufs=6))
        csbpool = ctx.enter_context(tc.tile_pool(name="csb", bufs=3))
        selpool = ctx.enter_context(tc.tile_pool(name="sel", bufs=3))
        outpool = ctx.enter_context(tc.tile_pool(name="outb", bufs=2))
        psg = ctx.enter_context(tc.tile_pool(name="psg", bufs=2, space="PSUM"))
        psc = ctx.enter_context(tc.tile_pool(name="psc", bufs=1, space="PSUM"))
        pst = ctx.enter_context(tc.tile_pool(name="pst", bufs=2, space="PSUM"))

        ident = cpool.tile([128, 128], F32)
        make_identity(nc, ident[:])
        wb_sb = cpool.tile([65, R, 64], F32)
        nc.sync.dma_start(wb_sb[:], wb[:])
        b_sb = cpool.tile([1, R, 64], F32)
        nc.sync.dma_start(b_sb[:], bvec[:])
        ones_sb = cpool.tile([1, 128], F32)
        nc.sync.dma_start(ones_sb[:], ones_in[:])
        iota_sb = cpool.tile([128, NQ], F32)
        nc.sync.dma_start(iota_sb[:], iota_in[:])
        pos_sb = cpool.tile([128, NT], F32)
        nc.sync.dma_start(pos_sb[:], pos_in[:])
        bulk_idx_sb = cpool.tile([128, NBULKC], I16)
        nc.sync.dma_start(bulk_idx_sb[:], bulk_idx[:])
        uniq_idx_sb = cpool.tile([128, NT_U * NQ // 16], I16)
        nc.sync.dma_start(uniq_idx_sb[:], uniq_idx[:])

        uslots = NT_U * NQ // 128
        uniq_sb = cpool.tile([128, uslots, D], F32)
        ucols = UBLK // 16
        for ub in range(n_ublk):
            nc.gpsimd.dma_gather(
                out_ap=uniq_sb[:, ub * (UBLK // 128):(ub + 1) * (UBLK // 128), :],
                in_ap=u_shard[:],
                idxs_ap=uniq_idx_sb[:, ub * ucols:(ub + 1) * ucols],
                num_idxs=UBLK, num_idxs_reg=UBLK, elem_size=D,
                single_packet=False, queue_num=3)

        g_ps = None
        it_sb = None
        t_global = 0
        bulk_col = 0
        for w in range(n_iwin):
            i_win_ap = i_tab[w * IWIN:min(NI, (w + 1) * IWIN), :]
            sec_t = 0
            for blk in range(n_bulk_blocks[w]):
                nt_here = min(BLK_TILES, sec_pad[w] - sec_t)
                if nt_here <= 0:
                    break
                ibig = bulkpool.tile([128, BLK_TILES, D], F32, tag="ibig")
                nc.gpsimd.dma_gather(
                    out_ap=ibig[:], in_ap=i_win_ap,
                    idxs_ap=bulk_idx_sb[:, bulk_col:bulk_col + BLK_TILES * 8],
                    num_idxs=BLK_TILES * 128, num_idxs_reg=BLK_TILES * 128,
                    elem_size=D, single_packet=False, queue_num=blk % 4)
                bulk_col += BLK_TILES * 8

                for g0 in range(0, nt_here, CGRP):
                    gt = t_global + g0
                    us = gt * NQ // 128
                    ut_ps = pst.tile([64, 2, 128], F32, tag="tps")
                    nc.tensor.transpose(out=ut_ps[:, 0, :], in_=uniq_sb[:, us, :],
                                        identity=ident[:])
                    ut_sb = itpool.tile([64, 128], F32, tag="utsb")
                    nc.scalar.copy(ut_sb[:], ut_ps[:, 0, :])
                    c_ps = psc.tile([64, R, 128], F32, tag="cps")
                    for r in range(R):
                        nc.tensor.matmul(c_ps[:, r, :], lhsT=wb_sb[0:64, r, :],
                                         rhs=ut_sb[:], start=True, stop=False)
                        nc.tensor.matmul(c_ps[:, r, :], lhsT=b_sb[:, r, :],
                                         rhs=ones_sb[:], start=False, stop=True)
                    c_sb = csbpool.tile([64, R, 128], F32, tag="csb")
                    nc.vector.tensor_copy(c_sb[:], c_ps[:])

                    for tt in range(CGRP):
                        t = gt + tt
                        s = g0 + tt
                        if tt % 2 == 0:
                            it_ps = pst.tile([64, 2, 128], F32, tag="tps")
                            nc.tensor.transpose(out=it_ps[:, 0, :],
                                                in_=ibig[:, s, :],
                                                identity=ident[:])
                            nc.tensor.transpose(out=it_ps[:, 1, :],
                                                in_=ibig[:, s + 1, :],
                                                identity=ident[:])
                            it_sb = itpool.tile([64, 2, 128], F32, tag="itsb")
                            if (t // 2) % 2 == 0:
                                nc.vector.tensor_copy(it_sb[:], it_ps[:])
                            else:
                                nc.scalar.copy(it_sb[:], it_ps[:])
                        tb = t % TBATCH
                        if tb == 0:
                            g_ps = psg.tile([128, TBATCH, 128], F32, tag="gps")
                        nc.tensor.matmul(
                            g_ps[:, tb, 0:R * NQ].rearrange(
                                "p (r q) -> p r q", r=R),
                            lhsT=it_sb[:, tt % 2, :],
                            rhs=c_sb[:, :, NQ * tt:NQ * (tt + 1)],
                            start=True, stop=True)

                        if tb == TBATCH - 1:
                            t0 = t - TBATCH + 1
                            onehot = selpool.tile([128, TBATCH, NQ], F32,
                                                  tag="onehot")
                            nc.vector.tensor_tensor(
                                out=onehot[:],
                                in0=pos_sb[:, t0:t0 + TBATCH]
                                    .to_broadcast([128, TBATCH, NQ]),
                                in1=_bc_mid(iota_sb[:], TBATCH, 1),
                                op=mybir.AluOpType.is_equal)
                            gsel = selpool.tile([128, TBATCH, R, NQ], F32,
                                                tag="gsel")
                            g_view = bass.AP(
                                g_ps[:].tensor, g_ps[:].offset,
                                [g_ps[:].ap[0], (128, TBATCH), (NQ, R),
                                 (1, NQ)])
                            nc.vector.tensor_tensor(
                                out=gsel[:], in0=g_view,
                                in1=_bc_mid(onehot[:], R, 2),
                                op=mybir.AluOpType.mult)
                            ob = outpool.tile([128, TBATCH, R], F32, tag="ob")
                            nc.vector.tensor_reduce(
                                out=ob[:], in_=gsel[:],
                                axis=mybir.AxisListType.X,
                                op=mybir.AluOpType.add)
                            ot_ps = pst.tile([128, 128], F32, tag="tps")
                            nc.tensor.transpose(
                                out=ot_ps[0:TBATCH * R, :],
                                in_=ob[:].rearrange("p a b -> p (a b)"),
                                identity=ident[:])
                            ot_sb = outpool.tile([TBATCH * R, 128], F32,
                                                 tag="otsb")
                            nc.scalar.copy(ot_sb[:], ot_ps[0:TBATCH * R, :])
                            nc.scalar.dma_start(
                                out_dev[t0 * R:(t0 + TBATCH) * R, :], ot_sb[:])
                t_global += nt_here
                sec_t += nt_here

    nc.compile()
    return nc


# ----------------------------------------------------------------- kernel()

def _run(u_features, i_features, edge_user, edge_item, W, b, trace=False):
    u_features = np.asarray(u_features, dtype=np.float32)
    i_features = np.asarray(i_features, dtype=np.float32)
    W = np.asarray(W, dtype=np.float32)
    b = np.asarray(b, dtype=np.float32)

    in_maps, reasm, meta = host_prep(u_features, i_features,
                                     edge_user, edge_item)
    nc = build_program(meta)

    wb_host = np.zeros((65, R, D), dtype=np.float32)
    wb_host[:64] = np.transpose(W, (2, 0, 1))   # [k, r, j] = W[r, j, k]
    wb_host[64] = b                              # row 64: b[r, j]
    iota16 = np.tile(np.arange(NQ, dtype=np.float32), (128, 1))
    ones128 = np.ones((1, 128), dtype=np.float32)
    b_host = np.ascontiguousarray(b.reshape(1, R, D))
    for m in in_maps:
        m["wb"] = wb_host
        m["bvec"] = b_host
        m["iota16"] = iota16
        m["ones_in"] = ones128

    res = run_bass_kernel_spmd(nc, in_maps, list(range(N_CORES)), trace=trace)

    E, NT = meta["E"], meta["NT"]
    out = np.zeros((E, R), dtype=np.float32)
    for c in range(N_CORES):
        od = res.results[c]["out_dev"].reshape(NT, R, 128)
        se = reasm[c]                           # [NT, 128]
        valid = se >= 0
        out[se[valid]] = od.transpose(0, 2, 1)[valid]
    return out, res


def kernel(u_features, i_features, edge_user, edge_item, W, b):
    out, _ = _run(u_features, i_features, edge_user, edge_item, W, b)
    return out



# revision 7
# speedup vs baseline: 2.5870x; 2.5870x over previous
"""BiDecoder edge kernel for Trainium2, 8-core SPMD (v2: bf16 compute,
transpose-mode item gather, per-edge select fused into a (parity,q) one-hot).

out[e, r] = (u[edge_user[e]] @ W[r].T + b[r]) . i[edge_item[e]]

Distribution: edges sharded by contiguous user ranges across 8 cores; each
core gets its own user-table slice (u_shard) plus the full item table.

Per-core algorithm:
  - host sorts the shard by user and packs 128-edge tiles with <= NQ=8
    unique users each; pair index item//2 fits int16, so no item windows;
  - item features: the bf16 item table is viewed as [NI/2, 128] rows; a
    transpose-mode SWDGE dma_gather of row m delivers a column
    [i_{2m}; i_{2m+1}] (two stacked feature vectors), so one 256B
    descriptor per edge; desc-gen is overlapped across all 4 SWDGE queues;
  - unique user rows: dma_gather f32 rows -> bf16 cast -> per-group PE
    transpose -> R matmuls with W^T give C[j, q] for 128 user slots;
    bias is folded in during the PSUM->SBUF C2 copy, into both parity
    halves (C2[0:64,(r,0,q)] = C+b, C2[64:128,(r,1,q)] = C+b, rest 0);
  - per tile one bf16 matmul G[e, (r,p,q)] = I2_tile^T @ C2-slice; a DVE
    one-hot (code = parity*8 + q) multiply + reduce selects per edge;
  - PE transposes output batches so the DRAM write is dense.
"""
import numpy as np
from contextlib import ExitStack

import concourse.bacc as bacc
import concourse.bass as bass
import concourse.tile as tile
from concourse import mybir
from concourse.bass_utils import run_bass_kernel_spmd
from concourse.masks import make_identity

F32 = mybir.dt.float32
BF16 = mybir.dt.bfloat16
I16 = mybir.dt.int16

N_CORES = 8
D = 64
R = 5
NQ = 8             # max unique users per 128-edge tile
CGRP = 16          # tiles per C-group (CGRP*NQ = 128 unique slots)
TBATCH = 8         # tiles per select/output batch
BLK = 64           # tiles per item-gather block (8192 edges)
UBLK = 2048        # unique rows per uniq dma_gather call


def _bf16(x):
    import ml_dtypes
    return np.asarray(x, dtype=np.float32).astype(ml_dtypes.bfloat16)


# ----------------------------------------------------------------- host prep

def _prep_core(users, items, eidx):
    """Pack one core's edges (sorted by user) into 128-edge tiles with
    <= NQ unique users. Returns per-tile arrays."""
    order = np.argsort(users, kind="stable")
    users = users[order]; items = items[order]; eidx = eidx[order]

    n = len(users)
    newgrp = np.ones(n, dtype=bool)
    if n > 1:
        newgrp[1:] = users[1:] != users[:-1]
    grp_starts = np.nonzero(newgrp)[0]
    grp_ends = np.append(grp_starts[1:], n)

    slot_item, slot_eidx, slot_pos, tile_uniq = [], [], [], []
    ci, ce, cp, cuq = [], [], [], []

    def flush():
        nonlocal ci, ce, cp, cuq
        if not ci:
            return
        pi = ci[0]
        while len(ci) < 128:
            ci.append(pi); ce.append(-1); cp.append(0)
        uq = list(cuq)
        while len(uq) < NQ:
            uq.append(uq[0])
        slot_item.append(ci); slot_eidx.append(ce)
        slot_pos.append(cp); tile_uniq.append(uq)
        ci, ce, cp, cuq = [], [], [], []

    for gs, ge in zip(grp_starts, grp_ends):
        u = int(users[gs])
        pos = int(gs)
        while pos < ge:
            if ci and (len(ci) >= 128 or
                       (u not in cuq and len(cuq) >= NQ)):
                flush()
            if u not in cuq:
                cuq.append(u)
            p = cuq.index(u)
            take = min(int(ge) - pos, 128 - len(ci))
            ci.extend(items[pos:pos + take].tolist())
            ce.extend(eidx[pos:pos + take].tolist())
            cp.extend([p] * take)
            pos += take
    flush()

    return {
        "slot_item": np.array(slot_item, dtype=np.int64),
        "slot_eidx": np.array(slot_eidx, dtype=np.int64),
        "slot_pos": np.array(slot_pos, dtype=np.int64),
        "tile_uniq": np.array(tile_uniq, dtype=np.int64),
    }


def _wrap16(idx_i16):
    """[n] int16 (n%16==0) -> [128, n//16]: idx j -> row j%16, col j//16,
    16-row block replicated to 128 partitions."""
    n = len(idx_i16)
    w = idx_i16.reshape(n // 16, 16).T
    return np.ascontiguousarray(np.tile(w, (8, 1)))


def host_prep(u_features, i_features, edge_user, edge_item):
    E = len(edge_user)
    NU = u_features.shape[0]
    NI = i_features.shape[0]
    NPAIR = (NI + 1) // 2

    eu = np.asarray(edge_user, dtype=np.int64)
    ei = np.asarray(edge_item, dtype=np.int64)

    # shard by contiguous user ranges with ~equal edge counts
    order_u = np.argsort(eu, kind="stable")
    eus = eu[order_u]
    bounds = [0]
    for c in range(1, N_CORES):
        t = (E * c) // N_CORES
        while 0 < t < E and eus[t] == eus[t - 1]:
            t += 1
        bounds.append(min(t, E))
    bounds.append(E)

    cores = []
    for c in range(N_CORES):
        sl = order_u[bounds[c]:bounds[c + 1]]
        cores.append(_prep_core(eu[sl], ei[sl], sl.copy()))

    ushard_start, ushard_len = [], []
    for c in range(N_CORES):
        uq = cores[c]["tile_uniq"]
        lo = int(uq.min()) if uq.size else 0
        hi = int(uq.max()) + 1 if uq.size else 1
        ushard_start.append(lo); ushard_len.append(hi - lo)
    MAXU = max(ushard_len)
    assert MAXU <= 32767, f"user shard {MAXU} exceeds int16 range"

    NT = max(len(cores[c]["slot_item"]) for c in range(N_CORES))
    NT = ((NT + BLK - 1) // BLK) * BLK
    n_uniq_idx = ((NT * NQ + UBLK - 1) // UBLK) * UBLK
    n_ublk = n_uniq_idx // UBLK
    USLOTS = n_uniq_idx // 128

    # item pair table: row m = [i_bf[2m], i_bf[2m+1]] = the flat bf16 table
    i_bf = _bf16(i_features)
    if NPAIR * 2 != NI:
        i_bf = np.concatenate([i_bf, i_bf[-1:]], axis=0)
    it_pair = np.ascontiguousarray(i_bf.reshape(NPAIR, 2 * D))

    in_maps, reasm = [], []
    for c in range(N_CORES):
        d = cores[c]
        k = len(d["slot_item"])
        slot_item_p = np.zeros((NT, 128), dtype=np.int64)
        slot_pos_p = np.zeros((NT, 128), dtype=np.int64)
        tile_uniq_p = np.zeros((NT, NQ), dtype=np.int64)
        slot_eidx_p = np.full((NT, 128), -1, dtype=np.int64)
        slot_item_p[:k] = d["slot_item"]
        slot_pos_p[:k] = d["slot_pos"]
        tile_uniq_p[:k] = d["tile_uniq"]
        tile_uniq_p[k:] = ushard_start[c]
        slot_eidx_p[:k] = d["slot_eidx"]

        pair_idx = (slot_item_p // 2).astype(np.int16).ravel()
        pos16 = (slot_item_p % 2) * NQ + slot_pos_p    # [NT, 128]

        uq = np.zeros(n_uniq_idx, dtype=np.int64)
        uq[:NT * NQ] = (tile_uniq_p - ushard_start[c]).ravel()
        uniq_wrapped = _wrap16(uq.astype(np.int16))

        ush = np.zeros((MAXU, D), dtype=np.float32)
        n_avail = min(MAXU, NU - ushard_start[c])
        ush[:n_avail] = u_features[ushard_start[c]:ushard_start[c] + n_avail]

        in_maps.append({
            "u_shard": ush,
            "it_pair": it_pair,
            "pair_idx": _wrap16(pair_idx),
            "uniq_idx": uniq_wrapped,
            "pos16": _bf16(np.ascontiguousarray(pos16.T)),
        })
        reasm.append(slot_eidx_p)

    meta = {"NT": NT, "n_ublk": n_ublk, "USLOTS": USLOTS,
            "MAXU": MAXU, "NPAIR": NPAIR, "E": E}
    return in_maps, reasm, meta


# ------------------------------------------------------------ device program

def build_program(meta):
    NT = meta["NT"]
    MAXU = meta["MAXU"]
    NPAIR = meta["NPAIR"]
    USLOTS = meta["USLOTS"]
    n_ublk = meta["n_ublk"]
    n_blk = NT // BLK

    nc = bacc.Bacc("TRN2", target_bir_lowering=False, debug=False,
                   num_devices=N_CORES, num_swdge_queues=4)

    u_shard = nc.dram_tensor("u_shard", [MAXU, D], F32, kind="ExternalInput").ap()
    it_pair = nc.dram_tensor("it_pair", [NPAIR, 2 * D], BF16,
                             kind="ExternalInput").ap()
    pair_idx = nc.dram_tensor("pair_idx", [128, NT * 8], I16,
                              kind="ExternalInput").ap()
    uniq_idx = nc.dram_tensor("uniq_idx", [128, n_ublk * UBLK // 16], I16,
                              kind="ExternalInput").ap()
    pos_in = nc.dram_tensor("pos16", [128, NT], BF16, kind="ExternalInput").ap()
    wbt = nc.dram_tensor("wbt", [64, R, 64], BF16, kind="ExternalInput").ap()
    bvec = nc.dram_tensor("bvec", [1, R, 64], BF16, kind="ExternalInput").ap()
    iota_in = nc.dram_tensor("iota16", [128, 2 * NQ], BF16,
                             kind="ExternalInput").ap()
    out_dev = nc.dram_tensor("out_dev", [128, NT * R], F32,
                             kind="ExternalOutput").ap()

    with tile.TileContext(nc) as tc, ExitStack() as ctx:
        cpool = ctx.enter_context(tc.tile_pool(name="const", bufs=1))
        idxpool = ctx.enter_context(tc.tile_pool(name="idx", bufs=4))
        upool = ctx.enter_context(tc.tile_pool(name="ustage", bufs=2))
        ipool = ctx.enter_context(tc.tile_pool(name="ipg", bufs=4))
        itpool = ctx.enter_context(tc.tile_pool(name="it", bufs=2))
        c2pool = ctx.enter_context(tc.tile_pool(name="c2p", bufs=3))
        selpool = ctx.enter_context(tc.tile_pool(name="sel", bufs=2))
        outpool = ctx.enter_context(tc.tile_pool(name="outb", bufs=2))
        psg = ctx.enter_context(tc.tile_pool(name="psg", bufs=2, space="PSUM"))
        psc = ctx.enter_context(tc.tile_pool(name="psc", bufs=1, space="PSUM"))
        pst = ctx.enter_context(tc.tile_pool(name="pst", bufs=2, space="PSUM"))

        ident = cpool.tile([128, 128], BF16)
        make_identity(nc, ident[:])
        wbt_sb = cpool.tile([64, R, 64], BF16)
        nc.sync.dma_start(wbt_sb[:], wbt[:])
        bvec_sb = cpool.tile([1, R, 64], BF16)
        nc.sync.dma_start(bvec_sb[:], bvec[:])
        ones_sb = cpool.tile([1, 128], BF16)
        nc.vector.memset(ones_sb[:], 1.0)
        iota_sb = cpool.tile([128, 2 * NQ], BF16)
        nc.sync.dma_start(iota_sb[:], iota_in[:])
        pos_sb = cpool.tile([128, NT], BF16)
        nc.sync.dma_start(pos_sb[:], pos_in[:])

        # unique user rows: gather f32, cast to bf16 resident
        uniq_bf = cpool.tile([128, USLOTS, D], BF16)
        for ub in range(n_ublk):
            uidx = idxpool.tile([128, UBLK // 16], I16, tag="uidx")
            nc.sync.dma_start(
                uidx[:], uniq_idx[:, ub * (UBLK // 16):(ub + 1) * (UBLK // 16)])
            stage = upool.tile([128, UBLK // 128, D], F32, tag="ustage")
            nc.gpsimd.dma_gather(
                out_ap=stage[:], in_ap=u_shard[:], idxs_ap=uidx[:],
                num_idxs=UBLK, num_idxs_reg=UBLK, elem_size=D,
                single_packet=False, queue_num=3)
            nc.scalar.copy(
                uniq_bf[:, ub * (UBLK // 128):(ub + 1) * (UBLK // 128), :],
                stage[:])

        # C2 buffers: static zero halves (written once, pool rotates 3 bufs)
        for k in range(3):
            c2p = c2pool.tile([128, R, 2, 128], BF16, tag="c2p")
            nc.vector.memset(c2p[:], 0.0)

        g_ps = None
        for blk in range(n_blk):
            pidx = idxpool.tile([128, BLK * 8], I16, tag="pidx")
            nc.sync.dma_start(
                pidx[:], pair_idx[:, blk * BLK * 8:(blk + 1) * BLK * 8])
            ipg = ipool.tile([128, 1, BLK * 128], BF16, tag="ipg")
            nc.gpsimd.dma_gather(
                out_ap=ipg[:], in_ap=it_pair[:], idxs_ap=pidx[:],
                num_idxs=BLK * 128, num_idxs_reg=BLK * 128, elem_size=2 * D,
                transpose=True, single_packet=False, queue_num=blk % 4)

            for g2 in range(BLK // CGRP):
                g = blk * (BLK // CGRP) + g2
                ut_ps = pst.tile([128, 128], BF16, tag="tps")
                nc.tensor.transpose(out=ut_ps[0:64, :], in_=uniq_bf[:, g, :],
                                    identity=ident[:])
                ut_sb = itpool.tile([64, 128], BF16, tag="utsb")
                nc.scalar.copy(ut_sb[:], ut_ps[0:64, :])
                c_ps = psc.tile([64, R, 128], F32, tag="cps")
                for r in range(R):
                    nc.tensor.matmul(c_ps[:, r, :], lhsT=wbt_sb[:, r, :],
                                     rhs=ut_sb[:], start=True, stop=False)
                    nc.tensor.matmul(c_ps[:, r, :], lhsT=bvec_sb[:, r, :],
                                     rhs=ones_sb[:], start=False, stop=True)
                c2p = c2pool.tile([128, R, 2, 128], BF16, tag="c2p")
                nc.scalar.copy(c2p[0:64, :, 0, :], c_ps[:])
                nc.scalar.copy(c2p[64:128, :, 1, :], c_ps[:])

                for tt in range(CGRP):
                    t = g * CGRP + tt
                    tb = t % TBATCH
                    if tb == 0:
                        g_ps = psg.tile([128, TBATCH, 128], F32, tag="gps")
                    e0 = (g2 * CGRP + tt) * 128
                    nc.tensor.matmul(
                        g_ps[:, tb, 0:R * 2 * NQ].rearrange(
                            "p (r x q) -> p r x q", r=R, x=2),
                        lhsT=ipg[:, 0, e0:e0 + 128],
                        rhs=c2p[:, :, :, tt * NQ:(tt + 1) * NQ],
                        start=True, stop=True)

                    if tb == TBATCH - 1:
                        t0 = t - TBATCH + 1
                        onehot = selpool.tile([128, TBATCH, 2 * NQ], BF16,
                                              tag="onehot")
                        nc.vector.tensor_tensor(
                            out=onehot[:],
                            in0=pos_sb[:, t0:t0 + TBATCH]
                                .to_broadcast([128, TBATCH, 2 * NQ]),
                            in1=bass.AP(iota_sb[:].tensor, iota_sb[:].offset,
                                        [iota_sb[:].ap[0], (0, TBATCH),
                                         (1, 2 * NQ)]),
                            op=mybir.AluOpType.is_equal)
                        gsel = selpool.tile([128, TBATCH, R, 2 * NQ], BF16,
                                            tag="gsel")
                        oh = onehot[:]
                        oh_bc = bass.AP(oh.tensor, oh.offset,
                                        [oh.ap[0], (2 * NQ, TBATCH), (0, R),
                                         (NQ, 2), (1, NQ)])
                        g_view = bass.AP(
                            g_ps[:].tensor, g_ps[:].offset,
                            [g_ps[:].ap[0], (128, TBATCH), (2 * NQ, R),
                             (NQ, 2), (1, NQ)])
                        nc.vector.tensor_tensor(
                            out=gsel[:].rearrange("p t r (x q) -> p t r x q",
                                                  x=2),
                            in0=g_view, in1=oh_bc,
                            op=mybir.AluOpType.mult)
                        ob = outpool.tile([128, TBATCH, R], F32, tag="ob")
                        nc.vector.tensor_reduce(
                            out=ob[:], in_=gsel[:],
                            axis=mybir.AxisListType.X,
                            op=mybir.AluOpType.add)
                        nc.scalar.dma_start(
                            out_dev[:, t0 * R:(t0 + TBATCH) * R],
                            ob[:].rearrange("p a b -> p (a b)"))

    nc.compile()
    return nc


# ----------------------------------------------------------------- kernel()

def _run(u_features, i_features, edge_user, edge_item, W, b, trace=False):
    u_features = np.asarray(u_features, dtype=np.float32)
    i_features = np.asarray(i_features, dtype=np.float32)
    W = np.asarray(W, dtype=np.float32)
    b = np.asarray(b, dtype=np.float32)

    in_maps, reasm, meta = host_prep(u_features, i_features,
                                     edge_user, edge_item)
    nc = build_program(meta)

    wbt_host = _bf16(np.transpose(W, (2, 0, 1)))     # [k, r, j] = W[r, j, k]
    bvec_host = _bf16(b.reshape(1, R, 64))
    iota16 = _bf16(np.tile(np.arange(2 * NQ, dtype=np.float32), (128, 1)))
    for m in in_maps:
        m["wbt"] = wbt_host
        m["bvec"] = bvec_host
        m["iota16"] = iota16

    res = run_bass_kernel_spmd(nc, in_maps, list(range(N_CORES)), trace=trace)

    E, NT = meta["E"], meta["NT"]
    out = np.zeros((E, R), dtype=np.float32)
    for c in range(N_CORES):
        od = res.results[c]["out_dev"].reshape(128, NT, R)
        se = reasm[c]                           # [NT, 128]
        valid = se >= 0
        out[se[valid]] = od.transpose(1, 0, 2)[valid]
    return out, res


def kernel(u_features, i_features, edge_user, edge_item, W, b):
    out, _ = _run(u_features, i_features, edge_user, edge_item, W, b)
    return out


# revision 12
# speedup vs baseline: 2.9620x; 1.1449x over previous
"""BiDecoder edge kernel for Trainium2, 8-core SPMD (v2: bf16 compute,
transpose-mode item gather, per-edge select fused into a (parity,q) one-hot).

out[e, r] = (u[edge_user[e]] @ W[r].T + b[r]) . i[edge_item[e]]

Distribution: edges sharded by contiguous user ranges across 8 cores; each
core gets its own user-table slice (u_shard) plus the full item table.

Per-core algorithm:
  - host sorts the shard by user and packs 128-edge tiles with <= NQ=8
    unique users each; pair index item//2 fits int16, so no item windows;
  - item features: the bf16 item table is viewed as [NI/2, 128] rows; a
    transpose-mode SWDGE dma_gather of row m delivers a column
    [i_{2m}; i_{2m+1}] (two stacked feature vectors), so one 256B
    descriptor per edge; desc-gen is overlapped across all 4 SWDGE queues;
  - unique user rows: dma_gather f32 rows -> bf16 cast -> per-group PE
    transpose -> R matmuls with W^T give C[j, q] for 128 user slots;
    bias is folded in during the PSUM->SBUF C2 copy, into both parity
    halves (C2[0:64,(r,0,q)] = C+b, C2[64:128,(r,1,q)] = C+b, rest 0);
  - per tile one bf16 matmul G[e, (r,p,q)] = I2_tile^T @ C2-slice; a DVE
    one-hot (code = parity*8 + q) multiply + reduce selects per edge;
  - PE transposes output batches so the DRAM write is dense.
"""
import numpy as np
from contextlib import ExitStack

import concourse.bacc as bacc
import concourse.bass as bass
import concourse.tile as tile
from concourse import mybir
from concourse.bass_utils import run_bass_kernel_spmd
from concourse.masks import make_identity

F32 = mybir.dt.float32
BF16 = mybir.dt.bfloat16
I16 = mybir.dt.int16

N_CORES = 8
D = 64
R = 5
NQ = 8             # max unique users per 128-edge tile
CGRP = 16          # tiles per C-group (CGRP*NQ = 128 unique slots)
TBATCH = 8         # tiles per select/output batch
BLK = 64           # tiles per item-gather block (8192 edges)
UBLK = 2048        # unique rows per uniq dma_gather call


def _bf16(x):
    import ml_dtypes
    return np.asarray(x, dtype=np.float32).astype(ml_dtypes.bfloat16)


# ----------------------------------------------------------------- host prep

def _prep_core(users, items, eidx):
    """Pack one core's edges (sorted by user) into 128-edge tiles with
    <= NQ unique users. Returns per-tile arrays."""
    order = np.argsort(users, kind="stable")
    users = users[order]; items = items[order]; eidx = eidx[order]

    n = len(users)
    newgrp = np.ones(n, dtype=bool)
    if n > 1:
        newgrp[1:] = users[1:] != users[:-1]
    grp_starts = np.nonzero(newgrp)[0]
    grp_ends = np.append(grp_starts[1:], n)

    slot_item, slot_eidx, slot_pos, tile_uniq = [], [], [], []
    ci, ce, cp, cuq = [], [], [], []

    def flush():
        nonlocal ci, ce, cp, cuq
        if not ci:
            return
        pi = ci[0]
        while len(ci) < 128:
            ci.append(pi); ce.append(-1); cp.append(0)
        uq = list(cuq)
        while len(uq) < NQ:
            uq.append(uq[0])
        slot_item.append(ci); slot_eidx.append(ce)
        slot_pos.append(cp); tile_uniq.append(uq)
        ci, ce, cp, cuq = [], [], [], []

    for gs, ge in zip(grp_starts, grp_ends):
        u = int(users[gs])
        pos = int(gs)
        while pos < ge:
            if ci and (len(ci) >= 128 or
                       (u not in cuq and len(cuq) >= NQ)):
                flush()
            if u not in cuq:
                cuq.append(u)
            p = cuq.index(u)
            take = min(int(ge) - pos, 128 - len(ci))
            ci.extend(items[pos:pos + take].tolist())
            ce.extend(eidx[pos:pos + take].tolist())
            cp.extend([p] * take)
            pos += take
    flush()

    return {
        "slot_item": np.array(slot_item, dtype=np.int64),
        "slot_eidx": np.array(slot_eidx, dtype=np.int64),
        "slot_pos": np.array(slot_pos, dtype=np.int64),
        "tile_uniq": np.array(tile_uniq, dtype=np.int64),
    }


def _wrap16(idx_i16):
    """[n] int16 (n%16==0) -> [128, n//16]: idx j -> row j%16, col j//16,
    16-row block replicated to 128 partitions."""
    n = len(idx_i16)
    w = idx_i16.reshape(n // 16, 16).T
    return np.ascontiguousarray(np.tile(w, (8, 1)))


def host_prep(u_features, i_features, edge_user, edge_item):
    E = len(edge_user)
    NU = u_features.shape[0]
    NI = i_features.shape[0]
    NPAIR = (NI + 1) // 2

    eu = np.asarray(edge_user, dtype=np.int64)
    ei = np.asarray(edge_item, dtype=np.int64)

    # shard by contiguous user ranges with ~equal edge counts
    order_u = np.argsort(eu, kind="stable")
    eus = eu[order_u]
    bounds = [0]
    for c in range(1, N_CORES):
        t = (E * c) // N_CORES
        while 0 < t < E and eus[t] == eus[t - 1]:
            t += 1
        bounds.append(min(t, E))
    bounds.append(E)

    cores = []
    for c in range(N_CORES):
        sl = order_u[bounds[c]:bounds[c + 1]]
        cores.append(_prep_core(eu[sl], ei[sl], sl.copy()))

    ushard_start, ushard_len = [], []
    for c in range(N_CORES):
        uq = cores[c]["tile_uniq"]
        lo = int(uq.min()) if uq.size else 0
        hi = int(uq.max()) + 1 if uq.size else 1
        ushard_start.append(lo); ushard_len.append(hi - lo)
    MAXU = max(ushard_len)
    assert MAXU <= 32767, f"user shard {MAXU} exceeds int16 range"

    NT = max(len(cores[c]["slot_item"]) for c in range(N_CORES))
    NT = ((NT + BLK - 1) // BLK) * BLK
    n_uniq_idx = ((NT * NQ + UBLK - 1) // UBLK) * UBLK
    n_ublk = n_uniq_idx // UBLK
    USLOTS = n_uniq_idx // 128

    # item pair table: row m = [i_bf[2m], i_bf[2m+1]] = the flat bf16 table
    i_bf = _bf16(i_features)
    if NPAIR * 2 != NI:
        i_bf = np.concatenate([i_bf, i_bf[-1:]], axis=0)
    it_pair = np.ascontiguousarray(i_bf.reshape(NPAIR, 2 * D))

    in_maps, reasm = [], []
    for c in range(N_CORES):
        d = cores[c]
        k = len(d["slot_item"])
        slot_item_p = np.zeros((NT, 128), dtype=np.int64)
        slot_pos_p = np.zeros((NT, 128), dtype=np.int64)
        tile_uniq_p = np.zeros((NT, NQ), dtype=np.int64)
        slot_eidx_p = np.full((NT, 128), -1, dtype=np.int64)
        slot_item_p[:k] = d["slot_item"]
        slot_pos_p[:k] = d["slot_pos"]
        tile_uniq_p[:k] = d["tile_uniq"]
        tile_uniq_p[k:] = ushard_start[c]
        slot_eidx_p[:k] = d["slot_eidx"]

        pair_idx = (slot_item_p // 2).astype(np.int16).ravel()
        pos16 = (slot_item_p % 2) * NQ + slot_pos_p    # [NT, 128]

        uq = np.zeros(n_uniq_idx, dtype=np.int64)
        uq[:NT * NQ] = (tile_uniq_p - ushard_start[c]).ravel()
        uniq_wrapped = _wrap16(uq.astype(np.int16))

        ush = np.zeros((MAXU, D), dtype=np.float32)
        n_avail = min(MAXU, NU - ushard_start[c])
        ush[:n_avail] = u_features[ushard_start[c]:ushard_start[c] + n_avail]

        in_maps.append({
            "u_shard": ush,
            "it_pair": it_pair,
            "pair_idx": _wrap16(pair_idx),
            "uniq_idx": uniq_wrapped,
            "pos16": _bf16(np.ascontiguousarray(pos16.T)),
        })
        reasm.append(slot_eidx_p)

    meta = {"NT": NT, "n_ublk": n_ublk, "USLOTS": USLOTS,
            "MAXU": MAXU, "NPAIR": NPAIR, "E": E}
    return in_maps, reasm, meta


# ------------------------------------------------------------ device program

def build_program(meta):
    NT = meta["NT"]
    MAXU = meta["MAXU"]
    NPAIR = meta["NPAIR"]
    USLOTS = meta["USLOTS"]
    n_ublk = meta["n_ublk"]
    n_blk = NT // BLK

    nc = bacc.Bacc("TRN2", target_bir_lowering=False, debug=False,
                   num_devices=N_CORES, num_swdge_queues=4)

    u_shard = nc.dram_tensor("u_shard", [MAXU, D], F32, kind="ExternalInput").ap()
    it_pair = nc.dram_tensor("it_pair", [NPAIR, 2 * D], BF16,
                             kind="ExternalInput").ap()
    pair_idx = nc.dram_tensor("pair_idx", [128, NT * 8], I16,
                              kind="ExternalInput").ap()
    uniq_idx = nc.dram_tensor("uniq_idx", [128, n_ublk * UBLK // 16], I16,
                              kind="ExternalInput").ap()
    pos_in = nc.dram_tensor("pos16", [128, NT], BF16, kind="ExternalInput").ap()
    wbt = nc.dram_tensor("wbt", [64, R, 64], BF16, kind="ExternalInput").ap()
    bvec = nc.dram_tensor("bvec", [1, R, 64], BF16, kind="ExternalInput").ap()
    iota_in = nc.dram_tensor("iota16", [128, 2 * NQ], BF16,
                             kind="ExternalInput").ap()
    out_dev = nc.dram_tensor("out_dev", [128, NT * R], F32,
                             kind="ExternalOutput").ap()

    with tile.TileContext(nc) as tc, ExitStack() as ctx:
        cpool = ctx.enter_context(tc.tile_pool(name="const", bufs=1))
        idxpool = ctx.enter_context(tc.tile_pool(name="idx", bufs=4))
        upool = ctx.enter_context(tc.tile_pool(name="ustage", bufs=2))
        ipool = ctx.enter_context(tc.tile_pool(name="ipg", bufs=4))
        itpool = ctx.enter_context(tc.tile_pool(name="it", bufs=3))
        c2pool = ctx.enter_context(tc.tile_pool(name="c2p", bufs=4))
        selpool = ctx.enter_context(tc.tile_pool(name="sel", bufs=3))
        outpool = ctx.enter_context(tc.tile_pool(name="outb", bufs=3))
        psg = ctx.enter_context(tc.tile_pool(name="psg", bufs=2, space="PSUM"))
        psc = ctx.enter_context(tc.tile_pool(name="psc", bufs=1, space="PSUM"))
        pst = ctx.enter_context(tc.tile_pool(name="pst", bufs=2, space="PSUM"))

        ident = cpool.tile([128, 128], BF16)
        make_identity(nc, ident[:])
        wbt_sb = cpool.tile([64, R, 64], BF16)
        nc.sync.dma_start(wbt_sb[:], wbt[:])
        bvec_sb = cpool.tile([1, R, 64], BF16)
        nc.sync.dma_start(bvec_sb[:], bvec[:])
        ones_sb = cpool.tile([1, 128], BF16)
        nc.vector.memset(ones_sb[:], 1.0)
        iota_sb = cpool.tile([128, 2 * NQ], BF16)
        nc.sync.dma_start(iota_sb[:], iota_in[:])
        pos_sb = cpool.tile([128, NT], BF16)
        nc.sync.dma_start(pos_sb[:], pos_in[:])

        # unique user rows: gather f32, cast to bf16 resident
        uniq_bf = cpool.tile([128, USLOTS, D], BF16)
        for ub in range(n_ublk):
            uidx = idxpool.tile([128, UBLK // 16], I16, tag="uidx")
            nc.sync.dma_start(
                uidx[:], uniq_idx[:, ub * (UBLK // 16):(ub + 1) * (UBLK // 16)])
            stage = upool.tile([128, UBLK // 128, D], F32, tag="ustage")
            nc.gpsimd.dma_gather(
                out_ap=stage[:], in_ap=u_shard[:], idxs_ap=uidx[:],
                num_idxs=UBLK, num_idxs_reg=UBLK, elem_size=D,
                single_packet=False, queue_num=3)
            nc.scalar.copy(
                uniq_bf[:, ub * (UBLK // 128):(ub + 1) * (UBLK // 128), :],
                stage[:])

        # C2 buffers: static zero halves (written once, pool rotates 4 bufs)
        for k in range(4):
            c2p = c2pool.tile([128, R, 2, 128], BF16, tag="c2p")
            nc.vector.memset(c2p[:], 0.0)

        g_ps = None
        n_grp = NT // CGRP

        def emit_cgroup(g):
            ut_ps = pst.tile([128, 128], BF16, tag="tps")
            nc.tensor.transpose(out=ut_ps[0:64, :], in_=uniq_bf[:, g, :],
                                identity=ident[:])
            ut_sb = itpool.tile([64, 128], BF16, tag="utsb")
            nc.scalar.copy(ut_sb[:], ut_ps[0:64, :])
            c_ps = psc.tile([64, R, 128], F32, tag="cps")
            for r in range(R):
                nc.tensor.matmul(c_ps[:, r, :], lhsT=wbt_sb[:, r, :],
                                 rhs=ut_sb[:], start=True, stop=False)
                nc.tensor.matmul(c_ps[:, r, :], lhsT=bvec_sb[:, r, :],
                                 rhs=ones_sb[:], start=False, stop=True)
            c2p = c2pool.tile([128, R, 2, 128], BF16, tag="c2p")
            nc.scalar.copy(c2p[0:64, :, 0, :], c_ps[:])
            nc.scalar.copy(c2p[64:128, :, 1, :], c_ps[:])
            return c2p

        c2p_next = emit_cgroup(0)
        for blk in range(n_blk):
            pidx = idxpool.tile([128, BLK * 8], I16, tag="pidx")
            nc.sync.dma_start(
                pidx[:], pair_idx[:, blk * BLK * 8:(blk + 1) * BLK * 8])
            ipg = ipool.tile([128, 1, BLK * 128], BF16, tag="ipg")
            nc.gpsimd.dma_gather(
                out_ap=ipg[:], in_ap=it_pair[:], idxs_ap=pidx[:],
                num_idxs=BLK * 128, num_idxs_reg=BLK * 128, elem_size=2 * D,
                transpose=True, single_packet=False, queue_num=blk % 4)

            for g2 in range(BLK // CGRP):
                g = blk * (BLK // CGRP) + g2
                c2p = c2p_next
                if g + 1 < n_grp:
                    c2p_next = emit_cgroup(g + 1)

                for tt in range(CGRP):
                    t = g * CGRP + tt
                    tb = t % TBATCH
                    if tb == 0:
                        g_ps = psg.tile([128, TBATCH, 128], F32, tag="gps")
                    e0 = (g2 * CGRP + tt) * 128
                    nc.tensor.matmul(
                        g_ps[:, tb, 0:R * 2 * NQ].rearrange(
                            "p (r x q) -> p r x q", r=R, x=2),
                        lhsT=ipg[:, 0, e0:e0 + 128],
                        rhs=c2p[:, :, :, tt * NQ:(tt + 1) * NQ],
                        start=True, stop=True)

                    if tb == TBATCH - 1:
                        t0 = t - TBATCH + 1
                        onehot = selpool.tile([128, TBATCH, 2 * NQ], BF16,
                                              tag="onehot")
                        nc.vector.tensor_tensor(
                            out=onehot[:],
                            in0=pos_sb[:, t0:t0 + TBATCH]
                                .to_broadcast([128, TBATCH, 2 * NQ]),
                            in1=bass.AP(iota_sb[:].tensor, iota_sb[:].offset,
                                        [iota_sb[:].ap[0], (0, TBATCH),
                                         (1, 2 * NQ)]),
                            op=mybir.AluOpType.is_equal)
                        gsel = selpool.tile([128, TBATCH, R, 2 * NQ], BF16,
                                            tag="gsel")
                        oh = onehot[:]
                        oh_bc = bass.AP(oh.tensor, oh.offset,
                                        [oh.ap[0], (2 * NQ, TBATCH), (0, R),
                                         (NQ, 2), (1, NQ)])
                        g_view = bass.AP(
                            g_ps[:].tensor, g_ps[:].offset,
                            [g_ps[:].ap[0], (128, TBATCH), (2 * NQ, R),
                             (NQ, 2), (1, NQ)])
                        nc.vector.tensor_tensor(
                            out=gsel[:].rearrange("p t r (x q) -> p t r x q",
                                                  x=2),
                            in0=g_view, in1=oh_bc,
                            op=mybir.AluOpType.mult)
                        ob = outpool.tile([128, TBATCH, R], F32, tag="ob")
                        nc.vector.tensor_reduce(
                            out=ob[:], in_=gsel[:],
                            axis=mybir.AxisListType.X,
                            op=mybir.AluOpType.add)
                        nc.scalar.dma_start(
                            out_dev[:, t0 * R:(t0 + TBATCH) * R],
                            ob[:].rearrange("p a b -> p (a b)"))

    nc.compile()
    return nc


# ----------------------------------------------------------------- kernel()

def _run(u_features, i_features, edge_user, edge_item, W, b, trace=False):
    u_features = np.asarray(u_features, dtype=np.float32)
    i_features = np.asarray(i_features, dtype=np.float32)
    W = np.asarray(W, dtype=np.float32)
    b = np.asarray(b, dtype=np.float32)

    in_maps, reasm, meta = host_prep(u_features, i_features,
                                     edge_user, edge_item)
    nc = build_program(meta)

    wbt_host = _bf16(np.transpose(W, (2, 0, 1)))     # [k, r, j] = W[r, j, k]
    bvec_host = _bf16(b.reshape(1, R, 64))
    iota16 = _bf16(np.tile(np.arange(2 * NQ, dtype=np.float32), (128, 1)))
    for m in in_maps:
        m["wbt"] = wbt_host
        m["bvec"] = bvec_host
        m["iota16"] = iota16

    res = run_bass_kernel_spmd(nc, in_maps, list(range(N_CORES)), trace=trace)

    E, NT = meta["E"], meta["NT"]
    out = np.zeros((E, R), dtype=np.float32)
    for c in range(N_CORES):
        od = res.results[c]["out_dev"].reshape(128, NT, R)
        se = reasm[c]                           # [NT, 128]
        valid = se >= 0
        out[se[valid]] = od.transpose(1, 0, 2)[valid]
    return out, res


def kernel(u_features, i_features, edge_user, edge_item, W, b):
    out, _ = _run(u_features, i_features, edge_user, edge_item, W, b)
    return out


# revision 13
# speedup vs baseline: 3.1595x; 1.0667x over previous
"""BiDecoder edge kernel for Trainium2, 8-core SPMD (v2: bf16 compute,
transpose-mode item gather, per-edge select fused into a (parity,q) one-hot).

out[e, r] = (u[edge_user[e]] @ W[r].T + b[r]) . i[edge_item[e]]

Distribution: edges sharded by contiguous user ranges across 8 cores; each
core gets its own user-table slice (u_shard) plus the full item table.

Per-core algorithm:
  - host sorts the shard by user and packs 128-edge tiles with <= NQ=8
    unique users each; pair index item//2 fits int16, so no item windows;
  - item features: the bf16 item table is viewed as [NI/2, 128] rows; a
    transpose-mode SWDGE dma_gather of row m delivers a column
    [i_{2m}; i_{2m+1}] (two stacked feature vectors), so one 256B
    descriptor per edge; desc-gen is overlapped across all 4 SWDGE queues;
  - unique user rows: dma_gather f32 rows -> bf16 cast -> per-group PE
    transpose -> R matmuls with W^T give C[j, q] for 128 user slots;
    bias is folded in during the PSUM->SBUF C2 copy, into both parity
    halves (C2[0:64,(r,0,q)] = C+b, C2[64:128,(r,1,q)] = C+b, rest 0);
  - per tile one bf16 matmul G[e, (r,p,q)] = I2_tile^T @ C2-slice; a DVE
    one-hot (code = parity*8 + q) multiply + reduce selects per edge;
  - PE transposes output batches so the DRAM write is dense.
"""
import numpy as np
from contextlib import ExitStack

import concourse.bacc as bacc
import concourse.bass as bass
import concourse.tile as tile
from concourse import mybir
from concourse.bass_utils import run_bass_kernel_spmd
from concourse.masks import make_identity

F32 = mybir.dt.float32
BF16 = mybir.dt.bfloat16
I16 = mybir.dt.int16

N_CORES = 8
D = 64
R = 5
NQ = 8             # max unique users per 128-edge tile
CGRP = 16          # tiles per C-group (CGRP*NQ = 128 unique slots)
TBATCH = 8         # tiles per select/output batch
BLK = 64           # tiles per item-gather block (8192 edges)
UBLK = 2048        # unique rows per uniq dma_gather call


def _bf16(x):
    import ml_dtypes
    return np.asarray(x, dtype=np.float32).astype(ml_dtypes.bfloat16)


# ----------------------------------------------------------------- host prep

def _prep_core(users, items, eidx):
    """Pack one core's edges (sorted by user) into 128-edge tiles with
    <= NQ unique users. Returns per-tile arrays."""
    order = np.argsort(users, kind="stable")
    users = users[order]; items = items[order]; eidx = eidx[order]

    n = len(users)
    newgrp = np.ones(n, dtype=bool)
    if n > 1:
        newgrp[1:] = users[1:] != users[:-1]
    grp_starts = np.nonzero(newgrp)[0]
    grp_ends = np.append(grp_starts[1:], n)

    slot_item, slot_eidx, slot_pos, tile_uniq = [], [], [], []
    ci, ce, cp, cuq = [], [], [], []

    def flush():
        nonlocal ci, ce, cp, cuq
        if not ci:
            return
        pi = ci[0]
        while len(ci) < 128:
            ci.append(pi); ce.append(-1); cp.append(0)
        uq = list(cuq)
        while len(uq) < NQ:
            uq.append(uq[0])
        slot_item.append(ci); slot_eidx.append(ce)
        slot_pos.append(cp); tile_uniq.append(uq)
        ci, ce, cp, cuq = [], [], [], []

    for gs, ge in zip(grp_starts, grp_ends):
        u = int(users[gs])
        pos = int(gs)
        while pos < ge:
            if ci and (len(ci) >= 128 or
                       (u not in cuq and len(cuq) >= NQ)):
                flush()
            if u not in cuq:
                cuq.append(u)
            p = cuq.index(u)
            take = min(int(ge) - pos, 128 - len(ci))
            ci.extend(items[pos:pos + take].tolist())
            ce.extend(eidx[pos:pos + take].tolist())
            cp.extend([p] * take)
            pos += take
    flush()

    return {
        "slot_item": np.array(slot_item, dtype=np.int64),
        "slot_eidx": np.array(slot_eidx, dtype=np.int64),
        "slot_pos": np.array(slot_pos, dtype=np.int64),
        "tile_uniq": np.array(tile_uniq, dtype=np.int64),
    }


def _wrap16(idx_i16):
    """[n] int16 (n%16==0) -> [128, n//16]: idx j -> row j%16, col j//16,
    16-row block replicated to 128 partitions."""
    n = len(idx_i16)
    w = idx_i16.reshape(n // 16, 16).T
    return np.ascontiguousarray(np.tile(w, (8, 1)))


def host_prep(u_features, i_features, edge_user, edge_item):
    E = len(edge_user)
    NU = u_features.shape[0]
    NI = i_features.shape[0]
    NPAIR = (NI + 1) // 2

    eu = np.asarray(edge_user, dtype=np.int64)
    ei = np.asarray(edge_item, dtype=np.int64)

    # shard by contiguous user ranges with ~equal edge counts
    order_u = np.argsort(eu, kind="stable")
    eus = eu[order_u]
    bounds = [0]
    for c in range(1, N_CORES):
        t = (E * c) // N_CORES
        while 0 < t < E and eus[t] == eus[t - 1]:
            t += 1
        bounds.append(min(t, E))
    bounds.append(E)

    cores = []
    for c in range(N_CORES):
        sl = order_u[bounds[c]:bounds[c + 1]]
        cores.append(_prep_core(eu[sl], ei[sl], sl.copy()))

    ushard_start, ushard_len = [], []
    for c in range(N_CORES):
        uq = cores[c]["tile_uniq"]
        lo = int(uq.min()) if uq.size else 0
        hi = int(uq.max()) + 1 if uq.size else 1
        ushard_start.append(lo); ushard_len.append(hi - lo)
    MAXU = max(ushard_len)
    assert MAXU <= 32767, f"user shard {MAXU} exceeds int16 range"

    NT = max(len(cores[c]["slot_item"]) for c in range(N_CORES))
    NT = ((NT + BLK - 1) // BLK) * BLK
    n_uniq_idx = ((NT * NQ + UBLK - 1) // UBLK) * UBLK
    n_ublk = n_uniq_idx // UBLK
    USLOTS = n_uniq_idx // 128

    # item pair table: row m = [i_bf[2m], i_bf[2m+1]] = the flat bf16 table
    i_bf = _bf16(i_features)
    if NPAIR * 2 != NI:
        i_bf = np.concatenate([i_bf, i_bf[-1:]], axis=0)
    it_pair = np.ascontiguousarray(i_bf.reshape(NPAIR, 2 * D))

    in_maps, reasm = [], []
    for c in range(N_CORES):
        d = cores[c]
        k = len(d["slot_item"])
        slot_item_p = np.zeros((NT, 128), dtype=np.int64)
        slot_pos_p = np.zeros((NT, 128), dtype=np.int64)
        tile_uniq_p = np.zeros((NT, NQ), dtype=np.int64)
        slot_eidx_p = np.full((NT, 128), -1, dtype=np.int64)
        slot_item_p[:k] = d["slot_item"]
        slot_pos_p[:k] = d["slot_pos"]
        tile_uniq_p[:k] = d["tile_uniq"]
        tile_uniq_p[k:] = ushard_start[c]
        slot_eidx_p[:k] = d["slot_eidx"]

        pair_idx = (slot_item_p // 2).astype(np.int16).ravel()
        pos16 = (slot_item_p % 2) * NQ + slot_pos_p    # [NT, 128]

        uq = np.zeros(n_uniq_idx, dtype=np.int64)
        uq[:NT * NQ] = (tile_uniq_p - ushard_start[c]).ravel()
        uniq_wrapped = _wrap16(uq.astype(np.int16))

        ush = np.zeros((MAXU, D), dtype=np.float32)
        n_avail = min(MAXU, NU - ushard_start[c])
        ush[:n_avail] = u_features[ushard_start[c]:ushard_start[c] + n_avail]

        in_maps.append({
            "u_shard": ush,
            "it_pair": it_pair,
            "pair_idx": _wrap16(pair_idx),
            "uniq_idx": uniq_wrapped,
            "pos16": _bf16(np.ascontiguousarray(pos16.T)),
        })
        reasm.append(slot_eidx_p)

    meta = {"NT": NT, "n_ublk": n_ublk, "USLOTS": USLOTS,
            "MAXU": MAXU, "NPAIR": NPAIR, "E": E}
    return in_maps, reasm, meta


# ------------------------------------------------------------ device program

def build_program(meta):
    NT = meta["NT"]
    MAXU = meta["MAXU"]
    NPAIR = meta["NPAIR"]
    USLOTS = meta["USLOTS"]
    n_ublk = meta["n_ublk"]
    n_blk = NT // BLK

    nc = bacc.Bacc("TRN2", target_bir_lowering=False, debug=False,
                   num_devices=N_CORES, num_swdge_queues=4)

    u_shard = nc.dram_tensor("u_shard", [MAXU, D], F32, kind="ExternalInput").ap()
    it_pair = nc.dram_tensor("it_pair", [NPAIR, 2 * D], BF16,
                             kind="ExternalInput").ap()
    pair_idx = nc.dram_tensor("pair_idx", [128, NT * 8], I16,
                              kind="ExternalInput").ap()
    uniq_idx = nc.dram_tensor("uniq_idx", [128, n_ublk * UBLK // 16], I16,
                              kind="ExternalInput").ap()
    pos_in = nc.dram_tensor("pos16", [128, NT], BF16, kind="ExternalInput").ap()
    wbt = nc.dram_tensor("wbt", [65, R, 64], BF16, kind="ExternalInput").ap()

    iota_in = nc.dram_tensor("iota16", [128, 2 * NQ], BF16,
                             kind="ExternalInput").ap()
    out_dev = nc.dram_tensor("out_dev", [128, NT * R], F32,
                             kind="ExternalOutput").ap()

    with tile.TileContext(nc) as tc, ExitStack() as ctx:
        cpool = ctx.enter_context(tc.tile_pool(name="const", bufs=1))
        idxpool = ctx.enter_context(tc.tile_pool(name="idx", bufs=4))
        upool = ctx.enter_context(tc.tile_pool(name="ustage", bufs=2))
        ipool = ctx.enter_context(tc.tile_pool(name="ipg", bufs=4))
        itpool = ctx.enter_context(tc.tile_pool(name="it", bufs=3))
        c2pool = ctx.enter_context(tc.tile_pool(name="c2p", bufs=4))
        selpool = ctx.enter_context(tc.tile_pool(name="sel", bufs=3))
        outpool = ctx.enter_context(tc.tile_pool(name="outb", bufs=3))
        psg = ctx.enter_context(tc.tile_pool(name="psg", bufs=2, space="PSUM"))
        psc = ctx.enter_context(tc.tile_pool(name="psc", bufs=1, space="PSUM"))
        pst = ctx.enter_context(tc.tile_pool(name="pst", bufs=2, space="PSUM"))

        ident = cpool.tile([128, 128], BF16)
        make_identity(nc, ident[:])
        wbt_sb = cpool.tile([65, R, 64], BF16)
        nc.sync.dma_start(wbt_sb[:], wbt[:])

        iota_sb = cpool.tile([128, 2 * NQ], BF16)
        nc.sync.dma_start(iota_sb[:], iota_in[:])
        pos_sb = cpool.tile([128, NT], BF16)
        nc.sync.dma_start(pos_sb[:], pos_in[:])

        # unique user rows: gather f32, cast to bf16 resident
        uniq_bf = cpool.tile([128, USLOTS, D], BF16)
        for ub in range(n_ublk):
            uidx = idxpool.tile([128, UBLK // 16], I16, tag="uidx")
            nc.sync.dma_start(
                uidx[:], uniq_idx[:, ub * (UBLK // 16):(ub + 1) * (UBLK // 16)])
            stage = upool.tile([128, UBLK // 128, D], F32, tag="ustage")
            nc.gpsimd.dma_gather(
                out_ap=stage[:], in_ap=u_shard[:], idxs_ap=uidx[:],
                num_idxs=UBLK, num_idxs_reg=UBLK, elem_size=D,
                single_packet=False, queue_num=3)
            nc.scalar.copy(
                uniq_bf[:, ub * (UBLK // 128):(ub + 1) * (UBLK // 128), :],
                stage[:])

        # C2 buffers: static zero halves (written once, pool rotates 4 bufs)
        for k in range(4):
            c2p = c2pool.tile([128, R, 2, 128], BF16, tag="c2p")
            nc.vector.memset(c2p[:], 0.0)

        g_ps = None
        n_grp = NT // CGRP

        # ut tiles carry a constant ones row 64 for the bias fold
        for k in range(3):
            ut_init = itpool.tile([65, 128], BF16, tag="utsb")
            nc.vector.memset(ut_init[64:65, :], 1.0)

        def emit_ut(g):
            ut_ps = pst.tile([128, 128], BF16, tag="tps")
            nc.tensor.transpose(out=ut_ps[0:64, :], in_=uniq_bf[:, g, :],
                                identity=ident[:])
            ut_sb = itpool.tile([65, 128], BF16, tag="utsb")
            nc.scalar.copy(ut_sb[0:64, :], ut_ps[0:64, :])
            return ut_sb

        def emit_c(ut_sb):
            c_ps = psc.tile([64, R, 128], F32, tag="cps")
            for r in range(R):
                nc.tensor.matmul(c_ps[:, r, :], lhsT=wbt_sb[:, r, :],
                                 rhs=ut_sb[:], start=True, stop=True)
            c2p = c2pool.tile([128, R, 2, 128], BF16, tag="c2p")
            nc.scalar.copy(c2p[0:64, :, 0, :], c_ps[:])
            nc.scalar.copy(c2p[64:128, :, 1, :], c_ps[:])
            return c2p

        ut_q = [emit_ut(0), emit_ut(1)]
        c2p_next = emit_c(ut_q.pop(0))
        for blk in range(n_blk):
            pidx = idxpool.tile([128, BLK * 8], I16, tag="pidx")
            nc.sync.dma_start(
                pidx[:], pair_idx[:, blk * BLK * 8:(blk + 1) * BLK * 8])
            ipg = ipool.tile([128, 1, BLK * 128], BF16, tag="ipg")
            nc.gpsimd.dma_gather(
                out_ap=ipg[:], in_ap=it_pair[:], idxs_ap=pidx[:],
                num_idxs=BLK * 128, num_idxs_reg=BLK * 128, elem_size=2 * D,
                transpose=True, single_packet=False, queue_num=blk % 4)

            for g2 in range(BLK // CGRP):
                g = blk * (BLK // CGRP) + g2
                c2p = c2p_next
                if g + 2 < n_grp:
                    ut_q.append(emit_ut(g + 2))
                if g + 1 < n_grp:
                    c2p_next = emit_c(ut_q.pop(0))

                for tt in range(CGRP):
                    t = g * CGRP + tt
                    tb = t % TBATCH
                    if tb == 0:
                        g_ps = psg.tile([128, TBATCH, 128], F32, tag="gps")
                    e0 = (g2 * CGRP + tt) * 128
                    nc.tensor.matmul(
                        g_ps[:, tb, 0:R * 2 * NQ].rearrange(
                            "p (r x q) -> p r x q", r=R, x=2),
                        lhsT=ipg[:, 0, e0:e0 + 128],
                        rhs=c2p[:, :, :, tt * NQ:(tt + 1) * NQ],
                        start=True, stop=True)

                    if tb == TBATCH - 1:
                        t0 = t - TBATCH + 1
                        onehot = selpool.tile([128, TBATCH, 2 * NQ], BF16,
                                              tag="onehot")
                        nc.vector.tensor_tensor(
                            out=onehot[:],
                            in0=pos_sb[:, t0:t0 + TBATCH]
                                .to_broadcast([128, TBATCH, 2 * NQ]),
                            in1=bass.AP(iota_sb[:].tensor, iota_sb[:].offset,
                                        [iota_sb[:].ap[0], (0, TBATCH),
                                         (1, 2 * NQ)]),
                            op=mybir.AluOpType.is_equal)
                        gsel = selpool.tile([128, TBATCH, R, 2 * NQ], BF16,
                                            tag="gsel")
                        oh = onehot[:]
                        oh_bc = bass.AP(oh.tensor, oh.offset,
                                        [oh.ap[0], (2 * NQ, TBATCH), (0, R),
                                         (NQ, 2), (1, NQ)])
                        g_view = bass.AP(
                            g_ps[:].tensor, g_ps[:].offset,
                            [g_ps[:].ap[0], (128, TBATCH), (2 * NQ, R),
                             (NQ, 2), (1, NQ)])
                        nc.vector.tensor_tensor(
                            out=gsel[:].rearrange("p t r (x q) -> p t r x q",
                                                  x=2),
                            in0=g_view, in1=oh_bc,
                            op=mybir.AluOpType.mult)
                        ob = outpool.tile([128, TBATCH, R], F32, tag="ob")
                        nc.vector.tensor_reduce(
                            out=ob[:], in_=gsel[:],
                            axis=mybir.AxisListType.X,
                            op=mybir.AluOpType.add)
                        nc.scalar.dma_start(
                            out_dev[:, t0 * R:(t0 + TBATCH) * R],
                            ob[:].rearrange("p a b -> p (a b)"))

    nc.compile()
    return nc


# ----------------------------------------------------------------- kernel()

def _run(u_features, i_features, edge_user, edge_item, W, b, trace=False):
    u_features = np.asarray(u_features, dtype=np.float32)
    i_features = np.asarray(i_features, dtype=np.float32)
    W = np.asarray(W, dtype=np.float32)
    b = np.asarray(b, dtype=np.float32)

    in_maps, reasm, meta = host_prep(u_features, i_features,
                                     edge_user, edge_item)
    nc = build_program(meta)

    wbt_host = np.zeros((65, R, 64), dtype=np.float32)
    wbt_host[0:64] = np.transpose(W, (2, 0, 1))      # [k, r, j] = W[r, j, k]
    wbt_host[64] = b                                  # bias row (ones in ut)
    wbt_host = _bf16(wbt_host)
    iota16 = _bf16(np.tile(np.arange(2 * NQ, dtype=np.float32), (128, 1)))
    for m in in_maps:
        m["wbt"] = wbt_host
        m["iota16"] = iota16

    res = run_bass_kernel_spmd(nc, in_maps, list(range(N_CORES)), trace=trace)

    E, NT = meta["E"], meta["NT"]
    out = np.zeros((E, R), dtype=np.float32)
    for c in range(N_CORES):
        od = res.results[c]["out_dev"].reshape(128, NT, R)
        se = reasm[c]                           # [NT, 128]
        valid = se >= 0
        out[se[valid]] = od.transpose(1, 0, 2)[valid]
    return out, res


def kernel(u_features, i_features, edge_user, edge_item, W, b):
    out, _ = _run(u_features, i_features, edge_user, edge_item, W, b)
    return out
